# revision 22
# baseline (speedup 1.0000x reference)
"""Multi-head attention on 8 TRN2 NeuronCores.

Problem: x[2, 2048, 1024], w_qkv[1024, 3072], w_out[1024, 1024] (f32).
  qkv = x @ w_qkv; q,k,v per 16 heads of dim 64; softmax(q k^T / 8) v; out proj.

Sharding: 16 heads split 8 ways (one head-PAIR per core, both batches on
every core).  Each core computes q^T/k^T/v for its 2 heads over all
B*L = 4096 rows, runs attention, then an 8-rank AllToAll exchanges
(head-pair -> (batch, L/4-chunk)) so each core finishes the output
projection for its own 512 output rows with all 16 heads present.  The
AllToAll is split into two half-exchanges (one per local head): the first
fires halfway through attention and fully overlaps the remaining compute.

Layout trick: scores are computed TRANSPOSED (S^T[m, l] tiles) so softmax's
sum runs over the partition axis -- done for free by appending a ones-column
to v in the attn@v matmul (out rows = [o^T; colsums]).  exp() runs on the
Scalar engine straight out of PSUM with the 1/8 scale folded in (scores are
~N(0,1) so no max-subtraction is needed for fp32/bf16 stability).
Normalization happens on small transposed tiles (per-partition scalars) in
bf16, then transposes back.

Tail: the out-projection is split by head parity.  Even heads arrive with
the first AllToAll, so their half of the contraction (partials parked in
SBUF f32) runs during the second AllToAll's transfer window; a chain of
heater matmuls keeps the PE's HAM clock gate warm across the collective so
the odd-head half + final add run at full clock instead of 1.2 GHz.

Compute dtype bf16 (f32 accumulation in PSUM).
"""

import sys
import types

sys.path.insert(0, "/opt/trn_rl_repo")

import numpy as np
import ml_dtypes

import concourse.bass as bass
import concourse.mybir as mybir
import concourse.tile as tile
from concourse import bacc
from concourse import bass_utils
from concourse.masks import make_identity

# If the image's antenv lacks the axon_hooks module, run_bass_kernel_spmd's
# trace path (reachable via BASS_TRACE=1) would die on import.  Provide the
# registry so tracing degrades gracefully instead (hook stays None unless
# trn_boot registered one).
try:
    import antenv.axon_hooks  # noqa: F401
except ImportError:
    _hooks = types.ModuleType("antenv.axon_hooks")
    _hooks._hook = None
    _hooks.set_axon_ntff_profile_hook = (
        lambda h: setattr(_hooks, "_hook", h))
    _hooks.get_axon_ntff_profile_hook = lambda: _hooks._hook
    sys.modules["antenv.axon_hooks"] = _hooks

# Artifact upload needs bucket credentials; fall back to the local dir so a
# traced run in a sandboxed container still completes.
_orig_upload = bass_utils.upload_artifacts


def _safe_upload(tmpdir):
    try:
        return _orig_upload(tmpdir)
    except Exception:
        return tmpdir


bass_utils.upload_artifacts = _safe_upload

B, L, D, H, DH = 2, 2048, 1024, 16, 64
BL = B * L  # 4096
SCALE = DH ** -0.5
N_CORES = 8
BF16 = mybir.dt.bfloat16
F32 = mybir.dt.float32
Exp = mybir.ActivationFunctionType.Exp

# exp on two engines: ScalarE evaluates the spline exp; the DVE handles the
# last 4 of every 16 key-tiles with a one-op Schraudolph bit-trick --
# bf16(int16(st + B)) ~= exp(st/A).  q is pre-scaled by A so the scores
# PSUM is already in bits-space; a uniform bits offset is a global scale
# on the softmax weights and cancels in normalization, so only the ~1.8%
# rms sawtooth remains, on 4/16 of the keys (~0.9% on the output).
SCH_A = 128 * 1.4426950408889634 * SCALE      # 23.0831
SCH_B = 16248.636                             # host-calibrated, zero mean bias
SCH_SCALE = SCALE / SCH_A                     # ScalarE: exp(st * this)
DVE_MPS = (0, 1)                              # mp pairs handled by the DVE

KT = D // 128          # 8 k-tiles over the model dim
MT = L // 128          # 16 m-tiles per batch
LC = L // 512          # 4 l-chunks of 512 per batch
VT = BL // 128         # 32 v row-tiles over (b, l)
N_HEAT = 58            # heater matmuls bridging the second AllToAll


def _build():
    nc = bacc.Bacc("TRN2", target_bir_lowering=False, debug=False,
                   num_devices=N_CORES)
    xT_ext = nc.declare_dram_parameter("xT", [D, BL], BF16, isOutput=False)
    wqk_ext = nc.declare_dram_parameter("wqk", [D, 256], BF16, isOutput=False)
    wv_ext = nc.declare_dram_parameter("wv", [D, 128], BF16, isOutput=False)
    # w_out rows pre-permuted on the host: first 512 = even heads
    # (0,2,..,14), last 512 = odd heads -- so the contraction splits into
    # a half that only needs the first AllToAll and a half that needs the
    # second.
    wout_ext = nc.declare_dram_parameter("wout", [D, D], BF16, isOutput=False)
    out_ext = nc.declare_dram_parameter("out", [512, D], F32, isOutput=True)

    with tile.TileContext(nc) as tc:
        with (
            tc.tile_pool(name="big", bufs=1) as big,
            tc.tile_pool(name="pt", bufs=2) as ptp,
            tc.tile_pool(name="small", bufs=2) as small,
            tc.tile_pool(name="psum_st", bufs=3, space="PSUM") as pst,
            tc.tile_pool(name="psum_ov", bufs=1, space="PSUM") as pov,
            tc.tile_pool(name="psum_tr", bufs=1, space="PSUM") as ptr,
            tc.tile_pool(name="dram", bufs=1, space="DRAM") as dram,
        ):
            # ---- static SBUF tensors ----
            xT_t = [big.tile([128, BL], BF16, tag=f"xT{k}", name=f"xT{k}") for k in range(KT)]
            wqk_t = [big.tile([128, 256], BF16, tag=f"wqk{k}", name=f"wqk{k}") for k in range(KT)]
            wv_t = [big.tile([128, 128], BF16, tag=f"wv{k}", name=f"wv{k}") for k in range(KT)]
            for k in range(KT):
                nc.sync.dma_start(
                    xT_t[k][:, 0:512], xT_ext[k * 128:(k + 1) * 128, 0:512])
                nc.sync.dma_start(wqk_t[k][:], wqk_ext[k * 128:(k + 1) * 128, :])
            for c0, c1 in ((512, 1536), (1536, 2560), (2560, 3584),
                           (3584, 4096)):
                for k in range(KT):
                    nc.sync.dma_start(
                        xT_t[k][:, c0:c1],
                        xT_ext[k * 128:(k + 1) * 128, c0:c1])
            for k in range(KT):
                nc.sync.dma_start(wv_t[k][:], wv_ext[k * 128:(k + 1) * 128, :])
            # out-proj weights: queued behind the x/qkv loads so they stream
            # in during the projection phase, long before the tail needs them.
            wout_t = [big.tile([128, D], BF16, tag=f"xT{k}", name=f"wout{k}")
                      for k in range(KT)]
            for k in range(KT):
                nc.sync.dma_start(wout_t[k][:], wout_ext[k * 128:(k + 1) * 128, :])

            # Warm the HAM clock gate during the initial xT DMA wait:
            # back-to-back matmuls lift the PE to 2.4 GHz before the first
            # real matmul issues.  Source is the first q/k weight tile --
            # it lands ~1.5us into the kernel, several us before any
            # gpsimd-produced tile would be ready.  Output is a scratch
            # accumulator nobody reads.
            wsrc = big.tile([128, 512], BF16, tag="wsrc")
            nc.gpsimd.memset(wsrc[:], 0.25)
            warm = pst.tile([128, 512], F32, tag="st", name="warm")
            for i in range(45):
                nc.tensor.matmul(warm[:, 0:128], wqk_t[0][:, 0:128],
                                 wqk_t[0][:, 0:128],
                                 start=(i == 0), stop=(i == 44))

            ident_b = big.tile([128, 128], BF16, tag="ident_b")
            make_identity(nc, ident_b[:])

            # q^T and k^T per head, rows 0:64 = head dims, rows 64:128 = 0.
            # Zero-padding keeps the full PE array active so the HAM clock
            # gate stays at 2.4 GHz (half-array matmuls throttle to 1.2).
            qp_t = [[big.tile([128, BL], BF16, tag=f"qp{m}{h}", name=f"qp{m}{h}")
                     for h in range(2)] for m in range(2)]
            for m in range(2):
                for h in range(2):
                    nc.gpsimd.memset(qp_t[m][h][64:128, :], 0.0)
            # v: cols [h*128 : h*128+64] = head h, +64 = ones, rest zero
            v_t = [big.tile([128, 256], BF16, tag=f"v{t}", name=f"v{t}") for t in range(VT)]
            # final o^T for our 2 heads, all 4096 rows
            oT_f = big.tile([128, BL], BF16, tag="oT")

            # ---- QKV projection ----
            # All the wide q/k matmuls first (dense N=512 streams keep the
            # HAM busy through the tail of the xT DMA); the small N=128 v
            # matmuls afterwards, where they dovetail with early attention.
            def emit_qk_cols(ncols):
                for ncol in ncols:
                    for m in range(2):  # 0 -> q, 1 -> k
                        ps = pov.tile([128, 512], F32, tag="ov",
                                      name=f"qk_ps{ncol}_{m}")
                        for k in range(KT):
                            nc.tensor.matmul(
                                ps[:],
                                wqk_t[k][:, m * 128:(m + 1) * 128],
                                xT_t[k][:, ncol * 512:(ncol + 1) * 512],
                                start=(k == 0), stop=(k == KT - 1),
                            )
                        for h in range(2):
                            nc.vector.tensor_copy(
                                qp_t[m][h][0:64, ncol * 512:(ncol + 1) * 512],
                                ps[h * 64:(h + 1) * 64, :])

            def emit_v_tiles(vts):
                for t in vts:
                    ps = ptr.tile([128, 128], F32, tag="tr",
                                  name=f"v_ps{t}")
                    for k in range(KT):
                        nc.tensor.matmul(
                            ps[:],
                            xT_t[k][:, t * 128:(t + 1) * 128],
                            wv_t[k][:],
                            start=(k == 0), stop=(k == KT - 1),
                        )
                    vv = v_t[t][:].rearrange("p (h c) -> p h c", h=2)
                    nc.gpsimd.memset(vv[:, :, 65:128], 0.0)
                    nc.vector.tensor_copy(
                        vv[:, :, 0:64],
                        ps[:].rearrange("p (h c) -> p h c", h=2))
                    nc.gpsimd.memset(vv[:, :, 64:65], 1.0)

            # ---- attention, one (batch, head) unit at a time ----
            # hl outermost: after all hl=0 units, half of oT_f (rows 0:64)
            # is final and its AllToAll overlaps the hl=1 attention.
            cc_in = [dram.tile([N_CORES, 64, 512], BF16, name=f"cc_in{i}")
                     for i in range(2)]
            cc_out = [dram.tile([N_CORES, 64, 512], BF16, name=f"cc_out{i}")
                      for i in range(2)]
            # received head data, split by parity: ogT_e[k] rows = heads
            # (4k, 4k+2) for my 512 queries; ogT_o[k] = heads (4k+1, 4k+3).
            ogT_e = [big.tile([128, 512], BF16, tag=f"wqk{k}", name=f"ogTe{k}")
                     for k in range(KT // 2)]
            ogT_o = [big.tile([128, 512], BF16, tag=f"wqk{k + 4}", name=f"ogTo{k}")
                     for k in range(KT // 2)]

            def emit_attn_unit(hl, b, filler=None):
                hs = slice(hl * 64, (hl + 1) * 64)
                for lc in range(LC):
                    ls = slice(b * L + lc * 512, b * L + (lc + 1) * 512)
                    pt = ptp.tile([128, MT, 512], BF16, tag="pt")
                    ov = pov.tile([128, 512], F32, tag="ov")
                    # S^T m-tile pair per PSUM tile so exp runs at
                    # FD=1024 (ScalarE per-instruction overhead is the
                    # bottleneck otherwise).
                    for mp in range(MT // 2):
                        st = pst.tile([128, 1024], F32, tag="st")
                        for h2 in range(2):
                            mt = 2 * mp + h2
                            nc.tensor.matmul(
                                st[:, h2 * 512:(h2 + 1) * 512],
                                qp_t[1][hl][:, b * L + mt * 128:
                                            b * L + (mt + 1) * 128],
                                qp_t[0][hl][:, ls],
                                start=True, stop=True,
                            )
                        nc.scalar.activation(
                            pt[:, 2 * mp:2 * mp + 2, :], st[:],
                            Exp, scale=SCALE)
                    for mt in range(MT):
                        nc.tensor.matmul(
                            ov[:],
                            v_t[b * MT + mt][:, hl * 128:(hl + 1) * 128],
                            pt[:, mt, :],
                            start=(mt == 0), stop=(mt == MT - 1),
                        )
                    # normalize via small bf16 transposes: batch the four
                    # 128-col blocks into one PSUM tile, one reciprocal
                    # over the 4 denominator columns, per-block scalar
                    # multiply, transpose back, single copy out.
                    if filler is not None:
                        filler(lc)
                    ovs = small.tile([128, 512], BF16, tag="ovs")
                    nc.vector.tensor_copy(ovs[0:65, :], ov[0:65, :])
                    trp = ptr.tile([128, 4, 66], BF16, tag="tr")
                    for j in range(4):
                        nc.tensor.transpose(
                            trp[:, j, 0:65], ovs[0:65, j * 128:(j + 1) * 128],
                            ident_b[0:65, 0:65])
                    rcp = small.tile([128, 4], F32, tag="rcp")
                    nc.vector.reciprocal(rcp[:], trp[:, :, 64])
                    onat = small.tile([128, 4, 64], BF16, tag="onat")
                    for j in range(4):
                        nc.vector.tensor_scalar(
                            onat[:, j, :], trp[:, j, 0:64],
                            rcp[:, j:j + 1], None,
                            mybir.AluOpType.mult)
                    trq = pov.tile([64, 4, 128], BF16, tag="ov")
                    for j in range(4):
                        nc.tensor.transpose(
                            trq[:, j, :], onat[:, j, :], ident_b[:])
                    nc.vector.tensor_copy(
                        oT_f[hs, b * L + lc * 512:b * L + (lc + 1) * 512],
                        trq[:].rearrange("p j c -> p (j c)"))

            def emit_a2a(hl):
                # half AllToAll: rows hl*64:(hl+1)*64 of oT_f are final
                hs = slice(hl * 64, (hl + 1) * 64)
                for j in range(N_CORES):
                    nc.sync.dma_start(cc_in[hl][j],
                                      oT_f[hs, j * 512:(j + 1) * 512])
                nc.gpsimd.collective_compute(
                    "AllToAll",
                    mybir.AluOpType.bypass,
                    ins=[cc_in[hl].opt()],
                    outs=[cc_out[hl].opt()],
                    replica_groups=[list(range(N_CORES))],
                )
                # head-parity regrouping: core j's hl-half is head 2j+hl.
                # even tile k collects heads (4k, 4k+2) = cores (2k, 2k+1)
                ogT = ogT_e if hl == 0 else ogT_o
                for k in range(KT // 2):
                    nc.sync.dma_start(
                        ogT[k][:],
                        cc_out[hl][2 * k:2 * k + 2].rearrange(
                            "a b c -> (a b) c"))

            # interleave: batch-1 qkv fills PE gaps of the first
            # (ScalarE-heavy) attention unit; each half-A2A overlaps
            # the next attention units.
            part = big.tile([128, 8, 512], F32, tag="part")

            def emit_phase1_group(g):
                # one even-head out-proj partial: 4 matmuls + copy to SBUF
                lt, nt = g // 2, g % 2
                ps = ptr.tile([128, 512], F32, tag="tr", name=f"ps1_{g}")
                for k in range(KT // 2):
                    nc.tensor.matmul(
                        ps[:],
                        ogT_e[k][:, lt * 128:(lt + 1) * 128],
                        wout_t[k][:, nt * 512:(nt + 1) * 512],
                        start=(k == 0), stop=(k == KT // 2 - 1),
                    )
                nc.scalar.copy(part[:, g, :], ps[:])

            def emit_miniheat(n):
                mh = pov.tile([128, 512], F32, tag="ov", name="mh")
                for i in range(n):
                    nc.tensor.matmul(mh[:], wsrc[:, 0:128], wsrc[:],
                                     start=(i == 0), stop=(i == n - 1))

            emit_qk_cols(range(0, 4))
            emit_qk_cols(range(4, 8))
            emit_v_tiles(range(0, MT))
            # batch-1 v tiles double as PE filler for the exp-paced first
            # attention units: 2 tiles after each l-chunk's matmul burst.
            emit_v_tiles(range(MT, 2 * MT))
            emit_attn_unit(0, 0)
            emit_attn_unit(0, 1)
            emit_a2a(0)
            emit_attn_unit(1, 0)
            emit_attn_unit(1, 1)
            # even-head out-proj partials: ready since the first AllToAll,
            # emitted here so they fill the second AllToAll's window.
            for g in range(8):
                emit_phase1_group(g)

            # Heater: keep the HAM clock gate warm across the second
            # AllToAll's ~18us transfer so phase 2 runs at full clock.
            # Reads oT_f (written by the last attention unit) so it cannot
            # be hoisted before the attention finishes; scratch accumulator.
            heat = pov.tile([128, 512], F32, tag="ov", name="heat")
            for i in range(N_HEAT):
                nc.tensor.matmul(heat[:], oT_f[:, 3584:3712],
                                 oT_f[:, 3584:4096],
                                 start=(i == 0), stop=(i == N_HEAT - 1))

            emit_a2a(1)

            # ---- out-proj phase 2: odd heads + merge with phase 1 ----
            for lt in range(4):
                for nt in range(2):
                    ps = pst.tile([128, 512], F32, tag="st")
                    for k in range(KT // 2):
                        nc.tensor.matmul(
                            ps[:],
                            ogT_o[k][:, lt * 128:(lt + 1) * 128],
                            wout_t[k + 4][:, nt * 512:(nt + 1) * 512],
                            start=(k == 0), stop=(k == KT // 2 - 1),
                        )
                    osb = small.tile([128, 512], F32, tag="osb")
                    nc.vector.tensor_tensor(
                        osb[:], ps[:], part[:, lt * 2 + nt, :],
                        mybir.AluOpType.add)
                    nc.sync.dma_start(
                        out_ext[lt * 128:(lt + 1) * 128,
                                nt * 512:(nt + 1) * 512],
                        osb[:])

    nc.compile()
    return nc


_NC_CACHE = None


def _get_nc():
    global _NC_CACHE
    if _NC_CACHE is None:
        _NC_CACHE = _build()
    return _NC_CACHE


# head-parity permutation of w_out rows: heads (0,2,..,14) then (1,3,..,15)
_WOUT_PERM = np.concatenate(
    [np.arange(h * DH, (h + 1) * DH)
     for h in list(range(0, H, 2)) + list(range(1, H, 2))])


def _make_in_maps(x, w_qkv, w_out):
    x = np.asarray(x, dtype=np.float32)
    w_qkv = np.asarray(w_qkv, dtype=np.float32)
    w_out = np.asarray(w_out, dtype=np.float32)
    bf = ml_dtypes.bfloat16
    xT = np.ascontiguousarray(
        x.transpose(2, 0, 1).reshape(D, BL)).astype(bf)
    wout_b = np.ascontiguousarray(w_out[_WOUT_PERM, :]).astype(bf)
    in_maps = []
    for c in range(N_CORES):
        cs = slice(c * 128, (c + 1) * 128)
        wqk_c = np.ascontiguousarray(
            np.concatenate([w_qkv[:, cs], w_qkv[:, D:][:, cs]], axis=1)
        ).astype(bf)
        wv_c = np.ascontiguousarray(w_qkv[:, 2 * D:][:, cs]).astype(bf)
        in_maps.append({"xT": xT, "wqk": wqk_c, "wv": wv_c, "wout": wout_b})
    return in_maps


def _run(x, w_qkv, w_out, trace=False):
    nc = _get_nc()
    in_maps = _make_in_maps(x, w_qkv, w_out)
    res = bass_utils.run_bass_kernel_spmd(
        nc, in_maps, list(range(N_CORES)), trace=trace)
    out = np.empty((B, L, D), dtype=np.float32)
    for c in range(N_CORES):
        out[c // 4, (c % 4) * 512:(c % 4 + 1) * 512, :] = \
            np.asarray(res.results[c]["out"])
    return out, res


def kernel(x, w_qkv, w_out):
    out, _ = _run(x, w_qkv, w_out, trace=False)
    return out


# revision 23
# speedup vs baseline: 1.2520x; 1.2520x over previous
"""Multi-head attention on 8 TRN2 NeuronCores.

Problem: x[2, 2048, 1024], w_qkv[1024, 3072], w_out[1024, 1024] (f32).
  qkv = x @ w_qkv; q,k,v per 16 heads of dim 64; softmax(q k^T / 8) v; out proj.

Sharding: 16 heads split 8 ways (one head-PAIR per core, both batches on
every core).  Each core computes q^T/k^T/v for its 2 heads over all
B*L = 4096 rows, runs attention, then an 8-rank AllToAll exchanges
(head-pair -> (batch, L/4-chunk)) so each core finishes the output
projection for its own 512 output rows with all 16 heads present.  The
AllToAll is split into two half-exchanges (one per local head): the first
fires halfway through attention and fully overlaps the remaining compute.

Layout trick: scores are computed TRANSPOSED (S^T[m, l] tiles) so softmax's
sum runs over the partition axis -- done for free by appending a ones-column
to v in the attn@v matmul (out rows = [o^T; colsums]).  exp() runs on the
Scalar engine straight out of PSUM with the 1/8 scale folded in (scores are
~N(0,1) so no max-subtraction is needed for fp32/bf16 stability).
Normalization happens on small transposed tiles (per-partition scalars) in
bf16, then transposes back.

Tail: the out-projection is split by head parity.  Even heads arrive with
the first AllToAll, so their half of the contraction (partials parked in
SBUF f32) runs during the second AllToAll's transfer window; a chain of
heater matmuls keeps the PE's HAM clock gate warm across the collective so
the odd-head half + final add run at full clock instead of 1.2 GHz.

Compute dtype bf16 (f32 accumulation in PSUM).
"""

import sys
import types

sys.path.insert(0, "/opt/trn_rl_repo")

import numpy as np
import ml_dtypes

import concourse.bass as bass
import concourse.mybir as mybir
import concourse.tile as tile
from concourse import bacc
from concourse import bass_utils
from concourse.masks import make_identity

# If the image's antenv lacks the axon_hooks module, run_bass_kernel_spmd's
# trace path (reachable via BASS_TRACE=1) would die on import.  Provide the
# registry so tracing degrades gracefully instead (hook stays None unless
# trn_boot registered one).
try:
    import antenv.axon_hooks  # noqa: F401
except ImportError:
    _hooks = types.ModuleType("antenv.axon_hooks")
    _hooks._hook = None
    _hooks.set_axon_ntff_profile_hook = (
        lambda h: setattr(_hooks, "_hook", h))
    _hooks.get_axon_ntff_profile_hook = lambda: _hooks._hook
    sys.modules["antenv.axon_hooks"] = _hooks

# Artifact upload needs bucket credentials; fall back to the local dir so a
# traced run in a sandboxed container still completes.
_orig_upload = bass_utils.upload_artifacts


def _safe_upload(tmpdir):
    try:
        return _orig_upload(tmpdir)
    except Exception:
        return tmpdir


bass_utils.upload_artifacts = _safe_upload

B, L, D, H, DH = 2, 2048, 1024, 16, 64
BL = B * L  # 4096
SCALE = DH ** -0.5
N_CORES = 8
BF16 = mybir.dt.bfloat16
F32 = mybir.dt.float32
Exp = mybir.ActivationFunctionType.Exp

# exp on two engines: ScalarE evaluates the spline exp; the DVE handles the
# last 4 of every 16 key-tiles with a one-op Schraudolph bit-trick --
# bf16(int16(st + B)) ~= exp(st/A).  q is pre-scaled by A so the scores
# PSUM is already in bits-space; a uniform bits offset is a global scale
# on the softmax weights and cancels in normalization, so only the ~1.8%
# rms sawtooth remains, on 4/16 of the keys (~0.9% on the output).
SCH_A = 128 * 1.4426950408889634 * SCALE      # 23.0831
SCH_B = 16248.636                             # host-calibrated, zero mean bias
SCH_SCALE = SCALE / SCH_A                     # ScalarE: exp(st * this)
DVE_MPS = (0, 1)                              # mp pairs handled by the DVE

KT = D // 128          # 8 k-tiles over the model dim
MT = L // 128          # 16 m-tiles per batch
LC = L // 512          # 4 l-chunks of 512 per batch
VT = BL // 128         # 32 v row-tiles over (b, l)
N_HEAT = 58            # heater matmuls bridging the second AllToAll


def _build():
    nc = bacc.Bacc("TRN2", target_bir_lowering=False, debug=False,
                   num_devices=N_CORES)
    xT_ext = nc.declare_dram_parameter("xT", [D, BL], BF16, isOutput=False)
    wqk_ext = nc.declare_dram_parameter("wqk", [D, 256], BF16, isOutput=False)
    wv_ext = nc.declare_dram_parameter("wv", [D, 128], BF16, isOutput=False)
    # w_out rows pre-permuted on the host: first 512 = even heads
    # (0,2,..,14), last 512 = odd heads -- so the contraction splits into
    # a half that only needs the first AllToAll and a half that needs the
    # second.
    wout_ext = nc.declare_dram_parameter("wout", [D, D], BF16, isOutput=False)
    out_ext = nc.declare_dram_parameter("out", [512, D], F32, isOutput=True)

    with tile.TileContext(nc) as tc:
        with (
            tc.tile_pool(name="big", bufs=1) as big,
            tc.tile_pool(name="pt", bufs=2) as ptp,
            tc.tile_pool(name="small", bufs=2) as small,
            tc.tile_pool(name="psum_st", bufs=3, space="PSUM") as pst,
            tc.tile_pool(name="psum_ov", bufs=1, space="PSUM") as pov,
            tc.tile_pool(name="psum_tr", bufs=1, space="PSUM") as ptr,
            tc.tile_pool(name="dram", bufs=1, space="DRAM") as dram,
        ):
            # ---- static SBUF tensors ----
            xT_t = [big.tile([128, BL], BF16, tag=f"xT{k}", name=f"xT{k}") for k in range(KT)]
            wqk_t = [big.tile([128, 256], BF16, tag=f"wqk{k}", name=f"wqk{k}") for k in range(KT)]
            wv_t = [big.tile([128, 128], BF16, tag=f"wv{k}", name=f"wv{k}") for k in range(KT)]
            for k in range(KT):
                nc.sync.dma_start(
                    xT_t[k][:, 0:512], xT_ext[k * 128:(k + 1) * 128, 0:512])
                nc.sync.dma_start(wqk_t[k][:], wqk_ext[k * 128:(k + 1) * 128, :])
            for cc in range(1, 8):
                for k in range(KT):
                    nc.sync.dma_start(
                        xT_t[k][:, cc * 512:(cc + 1) * 512],
                        xT_ext[k * 128:(k + 1) * 128, cc * 512:(cc + 1) * 512])
            for k in range(KT):
                nc.sync.dma_start(wv_t[k][:], wv_ext[k * 128:(k + 1) * 128, :])
            # out-proj weights: queued behind the x/qkv loads so they stream
            # in during the projection phase, long before the tail needs them.
            wout_t = [big.tile([128, D], BF16, tag=f"xT{k}", name=f"wout{k}")
                      for k in range(KT)]
            for k in range(KT):
                nc.sync.dma_start(wout_t[k][:], wout_ext[k * 128:(k + 1) * 128, :])

            # Warm the HAM clock gate during the initial xT DMA wait:
            # back-to-back matmuls lift the PE to 2.4 GHz before the first
            # real matmul issues.  Source is the first q/k weight tile --
            # it lands ~1.5us into the kernel, several us before any
            # gpsimd-produced tile would be ready.  Output is a scratch
            # accumulator nobody reads.
            wsrc = big.tile([128, 512], BF16, tag="wsrc")
            nc.gpsimd.memset(wsrc[:], 0.25)
            warm = pst.tile([128, 512], F32, tag="st", name="warm")
            for i in range(45):
                nc.tensor.matmul(warm[:, 0:128], wqk_t[0][:, 0:128],
                                 wqk_t[0][:, 0:128],
                                 start=(i == 0), stop=(i == 44))

            ident_b = big.tile([128, 128], BF16, tag="ident_b")
            make_identity(nc, ident_b[:])

            # q^T and k^T per head, rows 0:64 = head dims, rows 64:128 = 0.
            # Zero-padding keeps the full PE array active so the HAM clock
            # gate stays at 2.4 GHz (half-array matmuls throttle to 1.2).
            qp_t = [[big.tile([128, BL], BF16, tag=f"qp{m}{h}", name=f"qp{m}{h}")
                     for h in range(2)] for m in range(2)]
            for m in range(2):
                for h in range(2):
                    nc.gpsimd.memset(qp_t[m][h][64:128, :], 0.0)
            # v: cols [h*128 : h*128+64] = head h, +64 = ones, rest zero
            v_t = [big.tile([128, 256], BF16, tag=f"v{t}", name=f"v{t}") for t in range(VT)]
            # final o^T for our 2 heads, all 4096 rows
            oT_f = big.tile([128, BL], BF16, tag="oT")

            # ---- QKV projection ----
            # All the wide q/k matmuls first (dense N=512 streams keep the
            # HAM busy through the tail of the xT DMA); the small N=128 v
            # matmuls afterwards, where they dovetail with early attention.
            def emit_qk_cols(ncols):
                for ncol in ncols:
                    for m in range(2):  # 0 -> q, 1 -> k
                        ps = pov.tile([128, 512], F32, tag="ov",
                                      name=f"qk_ps{ncol}_{m}")
                        for k in range(KT):
                            nc.tensor.matmul(
                                ps[:],
                                wqk_t[k][:, m * 128:(m + 1) * 128],
                                xT_t[k][:, ncol * 512:(ncol + 1) * 512],
                                start=(k == 0), stop=(k == KT - 1),
                            )
                        for h in range(2):
                            nc.vector.tensor_copy(
                                qp_t[m][h][0:64, ncol * 512:(ncol + 1) * 512],
                                ps[h * 64:(h + 1) * 64, :])

            def emit_v_tiles(vts):
                for t in vts:
                    ps = ptr.tile([128, 128], F32, tag="tr",
                                  name=f"v_ps{t}")
                    for k in range(KT):
                        nc.tensor.matmul(
                            ps[:],
                            xT_t[k][:, t * 128:(t + 1) * 128],
                            wv_t[k][:],
                            start=(k == 0), stop=(k == KT - 1),
                        )
                    vv = v_t[t][:].rearrange("p (h c) -> p h c", h=2)
                    nc.gpsimd.memset(vv[:, :, 65:128], 0.0)
                    nc.vector.tensor_copy(
                        vv[:, :, 0:64],
                        ps[:].rearrange("p (h c) -> p h c", h=2))
                    nc.gpsimd.memset(vv[:, :, 64:65], 1.0)

            # ---- attention, one (batch, head) unit at a time ----
            # hl outermost: after all hl=0 units, half of oT_f (rows 0:64)
            # is final and its AllToAll overlaps the hl=1 attention.
            cc_in = [dram.tile([N_CORES, 64, 512], BF16, name=f"cc_in{i}")
                     for i in range(2)]
            cc_out = [dram.tile([N_CORES, 64, 512], BF16, name=f"cc_out{i}")
                      for i in range(2)]
            # received head data, split by parity: ogT_e[k] rows = heads
            # (4k, 4k+2) for my 512 queries; ogT_o[k] = heads (4k+1, 4k+3).
            ogT_e = [big.tile([128, 512], BF16, tag=f"wqk{k}", name=f"ogTe{k}")
                     for k in range(KT // 2)]
            ogT_o = [big.tile([128, 512], BF16, tag=f"wqk{k + 4}", name=f"ogTo{k}")
                     for k in range(KT // 2)]

            def emit_attn_unit(hl, b, filler=None):
                hs = slice(hl * 64, (hl + 1) * 64)
                for lc in range(LC):
                    ls = slice(b * L + lc * 512, b * L + (lc + 1) * 512)
                    pt = ptp.tile([128, MT, 512], BF16, tag="pt")
                    ov = pov.tile([128, 512], F32, tag="ov")
                    # S^T m-tile pair per PSUM tile so exp runs at
                    # FD=1024 (ScalarE per-instruction overhead is the
                    # bottleneck otherwise).
                    for mp in range(MT // 2):
                        st = pst.tile([128, 1024], F32, tag="st")
                        for h2 in range(2):
                            mt = 2 * mp + h2
                            nc.tensor.matmul(
                                st[:, h2 * 512:(h2 + 1) * 512],
                                qp_t[1][hl][:, b * L + mt * 128:
                                            b * L + (mt + 1) * 128],
                                qp_t[0][hl][:, ls],
                                start=True, stop=True,
                            )
                        nc.scalar.activation(
                            pt[:, 2 * mp:2 * mp + 2, :], st[:],
                            Exp, scale=SCALE)
                    for mt in range(MT):
                        nc.tensor.matmul(
                            ov[:],
                            v_t[b * MT + mt][:, hl * 128:(hl + 1) * 128],
                            pt[:, mt, :],
                            start=(mt == 0), stop=(mt == MT - 1),
                        )
                    # normalize via small bf16 transposes: batch the four
                    # 128-col blocks into one PSUM tile, one reciprocal
                    # over the 4 denominator columns, per-block scalar
                    # multiply, transpose back, single copy out.
                    if filler is not None:
                        filler(lc)
                    ovs = small.tile([128, 512], BF16, tag="ovs")
                    nc.vector.tensor_copy(ovs[0:65, :], ov[0:65, :])
                    trp = ptr.tile([128, 4, 66], BF16, tag="tr")
                    for j in range(4):
                        nc.tensor.transpose(
                            trp[:, j, 0:65], ovs[0:65, j * 128:(j + 1) * 128],
                            ident_b[0:65, 0:65])
                    rcp = small.tile([128, 4], F32, tag="rcp")
                    nc.vector.reciprocal(rcp[:], trp[:, :, 64])
                    onat = small.tile([128, 4, 64], BF16, tag="onat")
                    for j in range(4):
                        nc.vector.tensor_scalar(
                            onat[:, j, :], trp[:, j, 0:64],
                            rcp[:, j:j + 1], None,
                            mybir.AluOpType.mult)
                    trq = pov.tile([64, 4, 128], BF16, tag="ov")
                    for j in range(4):
                        nc.tensor.transpose(
                            trq[:, j, :], onat[:, j, :], ident_b[:])
                    nc.vector.tensor_copy(
                        oT_f[hs, b * L + lc * 512:b * L + (lc + 1) * 512],
                        trq[:].rearrange("p j c -> p (j c)"))

            def emit_a2a(hl):
                # half AllToAll: rows hl*64:(hl+1)*64 of oT_f are final
                hs = slice(hl * 64, (hl + 1) * 64)
                for j in range(N_CORES):
                    nc.sync.dma_start(cc_in[hl][j],
                                      oT_f[hs, j * 512:(j + 1) * 512])
                nc.gpsimd.collective_compute(
                    "AllToAll",
                    mybir.AluOpType.bypass,
                    ins=[cc_in[hl].opt()],
                    outs=[cc_out[hl].opt()],
                    replica_groups=[list(range(N_CORES))],
                )
                # head-parity regrouping: core j's hl-half is head 2j+hl.
                # even tile k collects heads (4k, 4k+2) = cores (2k, 2k+1)
                ogT = ogT_e if hl == 0 else ogT_o
                for k in range(KT // 2):
                    nc.sync.dma_start(
                        ogT[k][:],
                        cc_out[hl][2 * k:2 * k + 2].rearrange(
                            "a b c -> (a b) c"))

            # interleave: batch-1 qkv fills PE gaps of the first
            # (ScalarE-heavy) attention unit; each half-A2A overlaps
            # the next attention units.
            part = big.tile([128, 8, 512], F32, tag="part")

            def emit_phase1_group(g):
                # one even-head out-proj partial: 4 matmuls + copy to SBUF
                lt, nt = g // 2, g % 2
                ps = ptr.tile([128, 512], F32, tag="tr", name=f"ps1_{g}")
                for k in range(KT // 2):
                    nc.tensor.matmul(
                        ps[:],
                        ogT_e[k][:, lt * 128:(lt + 1) * 128],
                        wout_t[k][:, nt * 512:(nt + 1) * 512],
                        start=(k == 0), stop=(k == KT // 2 - 1),
                    )
                nc.scalar.copy(part[:, g, :], ps[:])

            def emit_miniheat(n):
                mh = pov.tile([128, 512], F32, tag="ov", name="mh")
                for i in range(n):
                    nc.tensor.matmul(mh[:], wsrc[:, 0:128], wsrc[:],
                                     start=(i == 0), stop=(i == n - 1))

            emit_qk_cols(range(0, 4))
            emit_qk_cols(range(4, 8))
            emit_v_tiles(range(0, MT))
            # batch-1 v tiles double as PE filler for the exp-paced first
            # attention units: 2 tiles after each l-chunk's matmul burst.
            emit_v_tiles(range(MT, 2 * MT))
            emit_attn_unit(0, 0)
            emit_attn_unit(0, 1)
            emit_a2a(0)
            emit_attn_unit(1, 0)
            emit_attn_unit(1, 1)
            # even-head out-proj partials: ready since the first AllToAll,
            # emitted here so they fill the second AllToAll's window.
            for g in range(8):
                emit_phase1_group(g)

            # Heater: keep the HAM clock gate warm across the second
            # AllToAll's ~18us transfer so phase 2 runs at full clock.
            # Reads oT_f (written by the last attention unit) so it cannot
            # be hoisted before the attention finishes; scratch accumulator.
            heat = pov.tile([128, 512], F32, tag="ov", name="heat")
            for i in range(N_HEAT):
                nc.tensor.matmul(heat[:], oT_f[:, 3584:3712],
                                 oT_f[:, 3584:4096],
                                 start=(i == 0), stop=(i == N_HEAT - 1))

            emit_a2a(1)

            # ---- out-proj phase 2: odd heads + merge with phase 1 ----
            for lt in range(4):
                for nt in range(2):
                    ps = pst.tile([128, 512], F32, tag="st")
                    for k in range(KT // 2):
                        nc.tensor.matmul(
                            ps[:],
                            ogT_o[k][:, lt * 128:(lt + 1) * 128],
                            wout_t[k + 4][:, nt * 512:(nt + 1) * 512],
                            start=(k == 0), stop=(k == KT // 2 - 1),
                        )
                    osb = small.tile([128, 512], F32, tag="osb")
                    nc.vector.tensor_tensor(
                        osb[:], ps[:], part[:, lt * 2 + nt, :],
                        mybir.AluOpType.add)
                    nc.sync.dma_start(
                        out_ext[lt * 128:(lt + 1) * 128,
                                nt * 512:(nt + 1) * 512],
                        osb[:])

    nc.compile()
    return nc


_NC_CACHE = None


def _get_nc():
    global _NC_CACHE
    if _NC_CACHE is None:
        _NC_CACHE = _build()
    return _NC_CACHE


# head-parity permutation of w_out rows: heads (0,2,..,14) then (1,3,..,15)
_WOUT_PERM = np.concatenate(
    [np.arange(h * DH, (h + 1) * DH)
     for h in list(range(0, H, 2)) + list(range(1, H, 2))])


def _make_in_maps(x, w_qkv, w_out):
    x = np.asarray(x, dtype=np.float32)
    w_qkv = np.asarray(w_qkv, dtype=np.float32)
    w_out = np.asarray(w_out, dtype=np.float32)
    bf = ml_dtypes.bfloat16
    xT = np.ascontiguousarray(
        x.transpose(2, 0, 1).reshape(D, BL)).astype(bf)
    wout_b = np.ascontiguousarray(w_out[_WOUT_PERM, :]).astype(bf)
    in_maps = []
    for c in range(N_CORES):
        cs = slice(c * 128, (c + 1) * 128)
        wqk_c = np.ascontiguousarray(
            np.concatenate([w_qkv[:, cs], w_qkv[:, D:][:, cs]], axis=1)
        ).astype(bf)
        wv_c = np.ascontiguousarray(w_qkv[:, 2 * D:][:, cs]).astype(bf)
        in_maps.append({"xT": xT, "wqk": wqk_c, "wv": wv_c, "wout": wout_b})
    return in_maps


def _run(x, w_qkv, w_out, trace=False):
    nc = _get_nc()
    in_maps = _make_in_maps(x, w_qkv, w_out)
    res = bass_utils.run_bass_kernel_spmd(
        nc, in_maps, list(range(N_CORES)), trace=trace)
    out = np.empty((B, L, D), dtype=np.float32)
    for c in range(N_CORES):
        out[c // 4, (c % 4) * 512:(c % 4 + 1) * 512, :] = \
            np.asarray(res.results[c]["out"])
    return out, res


def kernel(x, w_qkv, w_out):
    out, _ = _run(x, w_qkv, w_out, trace=False)
    return out
